# revision 13
# baseline (speedup 1.0000x reference)
"""Trainium2 Bass kernel for nn_AttentionBlock_56075093016781 (8 NeuronCores, SPMD).

Reference semantics (b=2, c=512, L=1024, num_heads=8):
  xn  = batchnorm(x) (stats over batch+length per channel) * gamma + beta
  qkv = w_qkv @ xn + b_qkv                  (1x1 conv over channels)
  layout quirk: qkv -> (b, 3*nh, hd, L) -> (b, hd, L, 3*nh); split q,k,v
    => 64 attention "heads" (the hd axis), feature dim 8 (the nh axis), T=1024
  w   = softmax(scale * q @ k^T) over keys, scale = (3*nh)**-0.5
  a   = w @ v ;  h[d*64+head, t] = a[head, t, d] ;  out = x + w_proj @ h + b_proj

Sharding: 8 cores = 2 batches x 4 head-groups of 16 heads. Each core computes
BN redundantly, its own q/k/v projections, attention for its 16 heads, and a
partial output projection over its 64 channels; host sums 4 bf16 partials per
batch in fp32 and adds the residual x + b_proj (exact fp32 on host).

v6 (from v5 @ ~178.5us): trace showed ~32us head (input DMA + serialized BN),
~38us tail (exposed AV of last quad + DMA-round-trip normalize + all of the
output projection + 2MB fp32 output DMA), and near-saturated ACT/DVE exp in
between. Changes:
  - AV(q) interleaves under quad q's own scores with a one-keyblock lag
    (crossing quad boundaries so PE never waits on fresh exps); the last
    quad's AV finishes with its scores -> tail is just normalize(3)+outproj
  - normalize: one-hot selector matmul broadcasts the denominator rows of
    a_sb to all 128 partitions in PSUM; reciprocal_approx_fast on the full
    [128,1024] tile; hout mult on GpSimd (quads 0-2) / DVE (quad 3). No
    DRAM round-trip, no small-gather DMAs.
  - head: x2 loaded as 4 full-width [128,2048] DMAs on 4 queues; xres input
    dropped (residual+bias on host); rsqrt Newton chain batched on [128,4];
    only kq(0)+v(0,1) before scores; kq(1..3) emitted as 4-MM quarters and
    v(2..7) as single-tt fillers inside quads 0-2
  - output partials bf16 (halves output DMA); host accumulates in fp32
  - exp pattern: full tiles only, 17 ACT / 15 DVE per quad
"""
import numpy as np
import ml_dtypes

import concourse.bass as bass
import concourse.bacc as bacc
import concourse.mybir as mybir
import concourse.tile as tile
from concourse.bass_utils import run_bass_kernel_spmd

F32 = mybir.dt.float32
BF16 = mybir.dt.bfloat16
I16 = mybir.dt.int16

B, C, L = 2, 512, 1024
NH = 8          # feature dim of each attention head (from num_heads)
HD = 64         # number of attention heads (head_dim axis of the quirky layout)
HEADS_PER_CORE = 16
N_CORES = 8
EPS = 1e-5

_CACHE = {}


def _build_nc():
    """Build the single-NeuronCore program (SPMD across 8 cores)."""
    nc = bacc.Bacc(None, target_bir_lowering=False)

    # ---- DRAM I/O ----
    x2_d = nc.dram_tensor("x2", [C, 2 * L], BF16, kind="ExternalInput")       # [c, b*L]
    gamma_d = nc.dram_tensor("gamma", [C], F32, kind="ExternalInput")
    beta_d = nc.dram_tensor("beta", [C], F32, kind="ExternalInput")
    wq_d = nc.dram_tensor("wqT", [C, 512], BF16, kind="ExternalInput")       # [c, padded qch] (scale folded)
    wk_d = nc.dram_tensor("wkT", [C, 512], BF16, kind="ExternalInput")
    wv_d = nc.dram_tensor("wvT", [C, 128], BF16, kind="ExternalInput")       # [c, vch compact]
    bq_d = nc.dram_tensor("bq", [512], F32, kind="ExternalInput")            # padded, scale folded
    bk_d = nc.dram_tensor("bk", [512], F32, kind="ExternalInput")
    wp_d = nc.dram_tensor("wpT", [512, 512], BF16, kind="ExternalInput")     # [padded c, o]
    sel_d = nc.dram_tensor("sel", [128, 128], BF16, kind="ExternalInput")    # denom broadcast selector
    out_d = nc.dram_tensor("out", [C, L], BF16, kind="ExternalOutput")

    with tile.TileContext(nc) as tc:
        with (
            tc.tile_pool(name="singles", bufs=1) as singles,
            tc.tile_pool(name="wt", bufs=12) as wtp,
            tc.tile_pool(name="norm", bufs=3) as normp,
            tc.tile_pool(name="outp", bufs=3) as outp,
            tc.tile_pool(name="psb", bufs=3, space="PSUM") as psb,
            tc.tile_pool(name="psav", bufs=1, space="PSUM") as psav,
        ):
            # ---- load x (full-width rows, 4 queues) and params ----
            xch = [[singles.tile([128, 1024], BF16, name=f"xc{i}_{k}") for k in range(2)]
                   for i in range(4)]
            dma_engs = [nc.sync, nc.gpsimd, nc.scalar]
            for ct in range(4):
                for k in range(2):
                    dma_engs[(2 * ct + k) % 3].dma_start(
                        xch[ct][k][:], x2_d[ct * 128:(ct + 1) * 128, k * 1024:(k + 1) * 1024])
            sel = singles.tile([128, 128], BF16, name="sel")
            nc.scalar.dma_start(sel[:], sel_d[:, :])
            gam = singles.tile([128, 4], F32, name="gam")
            bet = singles.tile([128, 4], F32, name="bet")
            nc.gpsimd.dma_start(gam[:], gamma_d.rearrange("(o p) -> p o", p=128))
            nc.gpsimd.dma_start(bet[:], beta_d.rearrange("(o p) -> p o", p=128))
            bqt = singles.tile([128, 4], F32, name="bqt")
            bkt = singles.tile([128, 4], F32, name="bkt")
            nc.scalar.dma_start(bqt[:], bq_d.rearrange("(o p) -> p o", p=128))
            nc.scalar.dma_start(bkt[:], bk_d.rearrange("(o p) -> p o", p=128))
            wq = [singles.tile([128, 512], BF16, name=f"wq{i}") for i in range(4)]
            wk = [singles.tile([128, 512], BF16, name=f"wk{i}") for i in range(4)]
            wv = [singles.tile([128, 128], BF16, name=f"wv{i}") for i in range(4)]
            wp = [singles.tile([128, 512], BF16, name=f"wp{i}") for i in range(4)]
            for ct in range(4):
                nc.sync.dma_start(wk[ct][:], wk_d[ct * 128:(ct + 1) * 128, :])
                nc.gpsimd.dma_start(wq[ct][:], wq_d[ct * 128:(ct + 1) * 128, :])
                nc.scalar.dma_start(wv[ct][:], wv_d[ct * 128:(ct + 1) * 128, :])

            # vsb ones/zeros after DMA emission so memsets don't block the
            # gpsimd DMA queue ahead of the x2 chunks
            vsb = [singles.tile([128, 16, 32], BF16, name=f"vsb{i}") for i in range(8)]
            for fb in range(8):
                nc.gpsimd.memset(vsb[fb][:, :, 8:9], 1.0)
                nc.gpsimd.memset(vsb[fb][:, :, 9:32], 0.0)

            # ---- BatchNorm stats (per half-chunk as x2 lands) ----
            mv4 = singles.tile([128, 4, 2], F32, name="mv4")
            for ct in range(4):
                stats = normp.tile([128, 4, 6], F32, tag="bnstats")
                for k in range(2):
                    xin = xch[ct][k].rearrange("p (s f) -> p s f", f=512)
                    for si in range(2):
                        nc.vector.bn_stats(out=stats[:, 2 * k + si, :], in_=xin[:, si, :])
                nc.vector.bn_aggr(out=mv4[:, ct, :], in_=stats[:])
            # rstd = sqrt(1/(var+eps)): recip-approx on DVE + ACT Sqrt
            # (Sqrt table load hidden in the idle head)
            r4 = singles.tile([128, 4], F32, name="r4")
            nc.vector.tensor_scalar(out=r4[:], in0=mv4[:, :, 1], scalar1=EPS,
                                    scalar2=None, op0=mybir.AluOpType.add)
            rr4 = singles.tile([128, 4], F32, name="rr4")
            with nc.allow_low_precision(reason="recip-approx 51 ULP, ample for bn rstd"):
                nc.vector.reciprocal_approx_fast(out=rr4[:], in_=r4[:])
            rstd4 = singles.tile([128, 4], F32, name="rstd4")
            nc.scalar.activation(out=rstd4[:], in_=rr4[:],
                                 func=mybir.ActivationFunctionType.Sqrt)
            s4 = singles.tile([128, 4], F32, name="s4")
            nc.vector.tensor_tensor(out=s4[:], in0=rstd4[:], in1=gam[:],
                                    op=mybir.AluOpType.mult)
            tm4 = singles.tile([128, 4], F32, name="tm4")
            nc.vector.tensor_tensor(out=tm4[:], in0=mv4[:, :, 0], in1=s4[:],
                                    op=mybir.AluOpType.mult)
            t4 = singles.tile([128, 4], F32, name="t4")
            nc.vector.tensor_tensor(out=t4[:], in0=bet[:], in1=tm4[:],
                                    op=mybir.AluOpType.subtract)
            # xn = s*x + t (own batch only): split ACT / DVE
            # all on DVE: bf16 in/out tensor_scalar hits the 2x perf mode
            xn = [singles.tile([128, L], BF16, name=f"xn{i}") for i in range(4)]
            for ct in range(4):
                nc.vector.tensor_scalar(out=xn[ct][:],
                                        in0=xch[ct][0][:],
                                        scalar1=s4[:, ct:ct + 1], scalar2=t4[:, ct:ct + 1],
                                        op0=mybir.AluOpType.mult, op1=mybir.AluOpType.add)

            # ---- k/q projections (padded head-block layout) ----
            kT = [singles.tile([128, L], BF16, name=f"kT{i}") for i in range(4)]
            qT = [singles.tile([128, L], BF16, name=f"qT{i}") for i in range(4)]
            kq_state = {}

            def emit_kq_quarter(mo, wi, nh_):
                # wi 0 = k, 1 = q; one nh-half = 4 accumulating MMs (~1.7us,
                # small enough that the exp backlog absorbs the PE insert)
                wmat = (wk, wq)[wi]
                key = (mo, wi)
                if nh_ == 0:
                    kq_state[key] = psb.tile([128, 1024], F32, tag="big", name="ps_kq")
                ps = kq_state[key]
                for kt in range(4):
                    nc.tensor.matmul(
                        ps[:, nh_ * 512:(nh_ + 1) * 512],
                        wmat[kt][:, mo * 128:(mo + 1) * 128],
                        xn[kt][:, nh_ * 512:(nh_ + 1) * 512],
                        start=(kt == 0), stop=(kt == 3))

            def emit_kq_cast(mo, wi):
                # split ACT/DVE so the psb buf frees in ~0.7us and neither
                # exp engine eats the full 1.2us cast
                bias_t, dst = ((bkt, kT), (bqt, qT))[wi]
                ps = kq_state.pop((mo, wi))
                nc.scalar.activation(out=dst[mo][:, 0:512], in_=ps[:, 0:512],
                                     func=mybir.ActivationFunctionType.Identity,
                                     bias=bias_t[:, mo:mo + 1])
                nc.vector.tensor_scalar(out=dst[mo][:, 512:1024], in0=ps[:, 512:1024],
                                        scalar1=bias_t[:, mo:mo + 1], scalar2=None,
                                        op0=mybir.AluOpType.add)

            def emit_kq(mo):
                for wi in range(2):
                    emit_kq_quarter(mo, wi, 0)
                    emit_kq_quarter(mo, wi, 1)
                    emit_kq_cast(mo, wi)

            v_state = {}

            def emit_v_mm(tt):
                # v projection block tt (flipped layout [t, vch]); bv is folded
                # out on the host (attention is affine in v: out += wp@bv)
                ps_full = psb.tile([128, 1024], F32, tag="big", name="psv")
                v_state[tt] = ps_full
                ps = ps_full[:, 0:128]
                for kt in range(4):
                    nc.tensor.matmul(ps[:], xn[kt][:, tt * 128:(tt + 1) * 128],
                                     wv[kt][:], start=(kt == 0), stop=(kt == 3))

            def emit_v_drain(tt):
                ps = v_state.pop(tt)[:, 0:128]
                nc.vector.tensor_copy(
                    vsb[tt][:, :, 0:8], ps.rearrange("p (h d) -> p h d", d=8))

            def emit_v(tt):
                emit_v_mm(tt)
                emit_v_drain(tt)

            emit_kq(0)
            emit_v(0)
            emit_v(1)

            # ---- attention: same-quad AV lag-1, phased normalize ----
            hout = [singles.tile([128, L], BF16, name=f"ho{i}") for i in range(4)]
            avs = {}
            wts = {}

            def emit_score_wave(qd, fb, nh_):
                for jp in range(2):
                    ps = psb.tile([128, 1024], F32, tag="big", name="ps_sc")
                    for side in range(2):
                        j = 2 * jp + side
                        nc.tensor.matmul(
                            ps[:, side * 512:(side + 1) * 512],
                            kT[qd][32 * j:32 * j + 32, fb * 128:(fb + 1) * 128],
                            qT[qd][32 * j:32 * j + 32, nh_ * 512:(nh_ + 1) * 512],
                            tile_position=(32 * j, 0))
                    emit_exp(qd, fb, jp, nh_, ps)

            def emit_exp(qd, fb, jp, nh_, ps):
                # every tile split at the ACT/DVE rate-balance point (both
                # ~610ns): max throughput AND every PSUM buf drains fast
                AC = 560
                wt = wtp.tile([128, 1024], BF16, tag="wt")
                nc.scalar.activation(out=wt[:, 0:AC], in_=ps[:, 0:AC],
                                     func=mybir.ActivationFunctionType.Exp)
                with nc.allow_low_precision(reason="schraudolph exp approx, validated"):
                    nc.vector.tensor_scalar(
                        out=wt.bitcast(I16)[:, AC:1024], in0=ps[:, AC:1024],
                        scalar1=184.66496, scalar2=16248.75,
                        op0=mybir.AluOpType.mult, op1=mybir.AluOpType.add)
                wts[(qd, fb, jp, nh_)] = wt

            def emit_av_half(qd, fb, nh_):
                if (qd, 0) not in avs:
                    avs[(qd, 0)] = psav.tile([128, 1024], F32, tag="av", name="av")
                av = avs[(qd, 0)]
                for j in range(4):
                    jp, side = j // 2, j % 2
                    nc.tensor.matmul(
                        av[32 * j:32 * j + 32, nh_ * 512:(nh_ + 1) * 512],
                        vsb[fb][:, 4 * qd + j, :],
                        wts[(qd, fb, jp, nh_)][:, side * 512:(side + 1) * 512],
                        start=(fb == 0), stop=(fb == 7),
                        tile_position=(0, 32 * j))
                if nh_ == 1:
                    for jp in range(2):
                        for nh2 in range(2):
                            del wts[(qd, fb, jp, nh2)]

            norm_state = {}

            def emit_norm_pool(qd):
                # a_sb copy (frees psav for the selector broadcast)
                av = avs.pop((qd, 0))
                a_sb = normp.tile([128, 1024], BF16, tag="asb")
                nc.scalar.activation(out=a_sb[:, 0:512], in_=av[:, 0:512],
                                     func=mybir.ActivationFunctionType.Copy)
                nc.vector.tensor_copy(a_sb[:, 512:1024], av[:, 512:1024])
                norm_state[qd] = a_sb

            def emit_norm_bcast(qd, last=False):
                # selector matmul broadcasts denom rows (32j+8) to all
                # partitions; full-tile reciprocal; hout = a_sb * recip
                a_sb = norm_state.pop(qd)
                rb_ps = psb.tile([128, 1024], F32, tag="big", name="rbps")
                for nh_ in range(2):
                    nc.tensor.matmul(rb_ps[:, nh_ * 512:(nh_ + 1) * 512],
                                     sel[:], a_sb[:, nh_ * 512:(nh_ + 1) * 512])
                rbr = normp.tile([128, 1024], F32, tag="rbr")
                with nc.allow_low_precision(reason="bf16 softmax denom recip, validated"):
                    nc.vector.reciprocal_approx_fast(out=rbr[:], in_=rb_ps[:])
                if last:
                    # th0 half on DVE first so outproj qd3/th0 can start early
                    nc.vector.tensor_tensor(out=hout[qd][:, 0:512], in0=a_sb[:, 0:512],
                                            in1=rbr[:, 0:512], op=mybir.AluOpType.mult)
                    nc.gpsimd.tensor_tensor(out=hout[qd][:, 512:1024], in0=a_sb[:, 512:1024],
                                            in1=rbr[:, 512:1024], op=mybir.AluOpType.mult)
                else:
                    nc.gpsimd.tensor_tensor(out=hout[qd][:], in0=a_sb[:], in1=rbr[:],
                                            op=mybir.AluOpType.mult)

            # filler schedule: (qd, fb, half) -> list of thunks
            fill = {}

            def add_fill(qd, fb, half, fn):
                fill.setdefault((qd, fb, half), []).append(fn)

            prefill = {}

            def add_prefill(qd, fb, fn):
                prefill.setdefault((qd, fb), []).append(fn)

            for tt in range(2, 8):                          # v(2..7) in quad 0
                add_fill(0, tt - 1, 1, (lambda t: lambda: emit_v_mm(t))(tt))
                add_prefill(0, tt, (lambda t: lambda: emit_v_drain(t))(tt))
            for mo in range(1, 4):                          # kq(1..3) in quads 0..2
                q = mo - 1
                for fb, wi, nh_ in [(2, 0, 0), (3, 0, 1), (5, 1, 0), (6, 1, 1)]:
                    add_fill(q, fb, 0,
                             (lambda m, w, n: lambda: emit_kq_quarter(m, w, n))(mo, wi, nh_))
                for fb, wi in [(4, 0), (7, 1)]:
                    add_prefill(q, fb,
                                (lambda m, w: lambda: emit_kq_cast(m, w))(mo, wi))

            def add_wp_load():
                for ct in range(4):
                    (nc.sync if ct % 2 == 0 else nc.gpsimd).dma_start(
                        wp[ct][:], wp_d[ct * 128:(ct + 1) * 128, :])
            add_fill(1, 4, 0, add_wp_load)

            for q in range(4):
                for fb in range(8):
                    # prefills (kq casts, v drains) run at cycle start so the
                    # psb bufs they hold free before PE needs them, and they
                    # sit ahead of this cycle's exps in the engine queues
                    for fn in prefill.get((q, fb), []):
                        fn()
                    emit_score_wave(q, fb, 0)
                    if fb > 0:
                        if fb == 1 and q > 0:
                            emit_norm_bcast(q - 1)
                        emit_av_half(q, fb - 1, 0)
                    elif q > 0:
                        # av(q-1,7) flush BEHIND the new quad's first scores
                        emit_av_half(q - 1, 7, 0)
                    for fn in fill.get((q, fb, 0), []):
                        fn()
                    emit_score_wave(q, fb, 1)
                    if fb > 0:
                        emit_av_half(q, fb - 1, 1)
                    elif q > 0:
                        emit_av_half(q - 1, 7, 1)
                        emit_norm_pool(q - 1)
                    for fn in fill.get((q, fb, 1), []):
                        fn()

            # ---- tail: AV(3,7), normalize(3), output projection ----
            emit_av_half(3, 7, 0)
            emit_av_half(3, 7, 1)
            emit_norm_pool(3)

            def emit_outproj_mo(mo, ps_full, lo, hi):
                for qd in range(lo, hi):
                    for th in range(2):
                        nc.tensor.matmul(ps_full[:, th * 512:(th + 1) * 512],
                                         wp[qd][:, mo * 128:(mo + 1) * 128],
                                         hout[qd][:, th * 512:(th + 1) * 512],
                                         start=(qd == 0), stop=(qd == 3))

            def finish_outproj(mo, th, ps):
                ot = outp.tile([128, 512], BF16, tag="ot")
                if (2 * mo + th) % 2 == 0:
                    nc.scalar.activation(out=ot[:], in_=ps[:],
                                         func=mybir.ActivationFunctionType.Copy)
                else:
                    nc.vector.tensor_copy(ot[:], ps[:])
                [nc.sync, nc.gpsimd, nc.scalar][(2 * mo + th) % 3].dma_start(
                    out_d[mo * 128:(mo + 1) * 128, th * 512:(th + 1) * 512], ot[:])

            ps_mo0 = psb.tile([128, 1024], F32, tag="big", name="pso")
            emit_outproj_mo(0, ps_mo0, 0, 3)
            emit_norm_bcast(3, last=True)
            ps_mo1 = psb.tile([128, 1024], F32, tag="big", name="pso")
            emit_outproj_mo(1, ps_mo1, 0, 3)
            emit_outproj_mo(0, ps_mo0, 3, 4)
            finish_outproj(0, 0, ps_mo0[:, 0:512])
            finish_outproj(0, 1, ps_mo0[:, 512:1024])
            emit_outproj_mo(1, ps_mo1, 3, 4)
            finish_outproj(1, 0, ps_mo1[:, 0:512])
            finish_outproj(1, 1, ps_mo1[:, 512:1024])
            for mo in range(2, 4):
                ps_full = psb.tile([128, 1024], F32, tag="big", name="pso")
                emit_outproj_mo(mo, ps_full, 0, 4)
                finish_outproj(mo, 0, ps_full[:, 0:512])
                finish_outproj(mo, 1, ps_full[:, 512:1024])

    nc.compile()
    return nc


def _prep_inputs(x, gamma, beta, w_qkv, b_qkv, w_proj, b_proj, num_heads):
    """Shard and lay out inputs for the 8 cores."""
    nh = int(num_heads)
    hd = C // nh
    scale = (3 * nh) ** (-0.5)
    wq_full, wk_full, wv_full = w_qkv[0:C], w_qkv[C:2 * C], w_qkv[2 * C:3 * C]
    bq_full, bk_full, bv_full = b_qkv[0:C], b_qkv[C:2 * C], b_qkv[2 * C:3 * C]

    bf16 = ml_dtypes.bfloat16
    sel = np.zeros((128, 128), np.float32)
    for m in range(128):
        sel[32 * (m // 32) + 8, m] = 1.0
    in_maps = []
    for core in range(N_CORES):
        bi = core // 4
        g = core % 4
        heads = list(range(HEADS_PER_CORE * g, HEADS_PER_CORE * (g + 1)))
        # x2: own batch first, other batch second (BN stats use both)
        x2 = np.concatenate([x[bi], x[1 - bi]], axis=1).astype(bf16)

        # padded q/k weight layouts: [c, 512], col 32*jl + d = channel d*hd + h
        wqT = np.zeros((C, 512), np.float32)
        wkT = np.zeros((C, 512), np.float32)
        bq = np.zeros(512, np.float32)
        bk = np.zeros(512, np.float32)
        wvT = np.zeros((C, 128), np.float32)
        wpT = np.zeros((512, C), np.float32)
        for jl, h in enumerate(heads):
            for d in range(nh):
                ch = d * hd + h
                wqT[:, 32 * jl + d] = wq_full[ch] * scale
                wkT[:, 32 * jl + d] = wk_full[ch]
                bq[32 * jl + d] = bq_full[ch] * scale
                bk[32 * jl + d] = bk_full[ch]
                wvT[:, 8 * jl + d] = wv_full[ch]
                wpT[32 * jl + d, :] = w_proj[:, ch]

        in_maps.append({
            "x2": np.ascontiguousarray(x2),
            "gamma": np.ascontiguousarray(gamma.astype(np.float32)),
            "beta": np.ascontiguousarray(beta.astype(np.float32)),
            "wqT": np.ascontiguousarray(wqT.astype(bf16)),
            "wkT": np.ascontiguousarray(wkT.astype(bf16)),
            "wvT": np.ascontiguousarray(wvT.astype(bf16)),
            "bq": bq, "bk": bk,
            "wpT": np.ascontiguousarray(wpT.astype(bf16)),
            "sel": np.ascontiguousarray(sel.astype(bf16)),
        })
    return in_maps


def kernel(x, gamma, beta, w_qkv, b_qkv, w_proj, b_proj, num_heads, _trace=False):
    x = np.asarray(x, dtype=np.float32)
    gamma = np.asarray(gamma, np.float32)
    beta = np.asarray(beta, np.float32)
    w_qkv = np.asarray(w_qkv, np.float32)
    b_qkv = np.asarray(b_qkv, np.float32)
    w_proj = np.asarray(w_proj, np.float32)
    b_proj = np.asarray(b_proj, np.float32)

    if "nc" not in _CACHE:
        _CACHE["nc"] = _build_nc()
    nc = _CACHE["nc"]
    in_maps = _prep_inputs(x, gamma, beta, w_qkv, b_qkv, w_proj, b_proj, num_heads)
    res = run_bass_kernel_spmd(nc, in_maps, core_ids=list(range(N_CORES)), trace=_trace)
    _CACHE["last_result"] = res

    # bias fold: attention is affine in v (softmax rows sum to 1), so the
    # v-bias flows through as a constant: out += w_proj @ bv + b_proj
    bvh = w_proj @ b_qkv[2 * C:3 * C]
    out = np.zeros((B, C, L), np.float32)
    for bi in range(B):
        acc = np.zeros((C, L), np.float32)
        for g in range(4):
            acc += np.asarray(res.results[bi * 4 + g]["out"]).astype(np.float32)
        out[bi] = acc + x[bi] + (b_proj + bvh)[:, None]
    return out


# revision 14
# speedup vs baseline: 1.1901x; 1.1901x over previous
"""Trainium2 Bass kernel for nn_AttentionBlock_56075093016781 (8 NeuronCores, SPMD).

Reference semantics (b=2, c=512, L=1024, num_heads=8):
  xn  = batchnorm(x) (stats over batch+length per channel) * gamma + beta
  qkv = w_qkv @ xn + b_qkv                  (1x1 conv over channels)
  layout quirk: qkv -> (b, 3*nh, hd, L) -> (b, hd, L, 3*nh); split q,k,v
    => 64 attention "heads" (the hd axis), feature dim 8 (the nh axis), T=1024
  w   = softmax(scale * q @ k^T) over keys, scale = (3*nh)**-0.5
  a   = w @ v ;  h[d*64+head, t] = a[head, t, d] ;  out = x + w_proj @ h + b_proj

Sharding: 8 cores = 2 batches x 4 head-groups of 16 heads. Each core computes
BN redundantly, its own q/k/v projections, attention for its 16 heads, and a
partial output projection over its 64 channels; host sums 4 bf16 partials per
batch in fp32 and adds the residual x + b_proj (exact fp32 on host).

v6 (from v5 @ ~178.5us): trace showed ~32us head (input DMA + serialized BN),
~38us tail (exposed AV of last quad + DMA-round-trip normalize + all of the
output projection + 2MB fp32 output DMA), and near-saturated ACT/DVE exp in
between. Changes:
  - AV(q) interleaves under quad q's own scores with a one-keyblock lag
    (crossing quad boundaries so PE never waits on fresh exps); the last
    quad's AV finishes with its scores -> tail is just normalize(3)+outproj
  - normalize: one-hot selector matmul broadcasts the denominator rows of
    a_sb to all 128 partitions in PSUM; reciprocal_approx_fast on the full
    [128,1024] tile; hout mult on GpSimd (quads 0-2) / DVE (quad 3). No
    DRAM round-trip, no small-gather DMAs.
  - head: x2 loaded as 4 full-width [128,2048] DMAs on 4 queues; xres input
    dropped (residual+bias on host); rsqrt Newton chain batched on [128,4];
    only kq(0)+v(0,1) before scores; kq(1..3) emitted as 4-MM quarters and
    v(2..7) as single-tt fillers inside quads 0-2
  - output partials bf16 (halves output DMA); host accumulates in fp32
  - exp pattern: full tiles only, 17 ACT / 15 DVE per quad
"""
import numpy as np
import ml_dtypes

import concourse.bass as bass
import concourse.bacc as bacc
import concourse.mybir as mybir
import concourse.tile as tile
from concourse.bass_utils import run_bass_kernel_spmd

F32 = mybir.dt.float32
BF16 = mybir.dt.bfloat16
I16 = mybir.dt.int16

B, C, L = 2, 512, 1024
NH = 8          # feature dim of each attention head (from num_heads)
HD = 64         # number of attention heads (head_dim axis of the quirky layout)
HEADS_PER_CORE = 16
N_CORES = 8
EPS = 1e-5

_CACHE = {}


def _build_nc():
    """Build the single-NeuronCore program (SPMD across 8 cores)."""
    nc = bacc.Bacc(None, target_bir_lowering=False)

    # ---- DRAM I/O ----
    x2_d = nc.dram_tensor("x2", [C, 2 * L], BF16, kind="ExternalInput")       # [c, b*L]
    gamma_d = nc.dram_tensor("gamma", [C], F32, kind="ExternalInput")
    beta_d = nc.dram_tensor("beta", [C], F32, kind="ExternalInput")
    wq_d = nc.dram_tensor("wqT", [C, 512], BF16, kind="ExternalInput")       # [c, padded qch] (scale folded)
    wk_d = nc.dram_tensor("wkT", [C, 512], BF16, kind="ExternalInput")
    wv_d = nc.dram_tensor("wvT", [C, 128], BF16, kind="ExternalInput")       # [c, vch compact]
    bq_d = nc.dram_tensor("bq", [512], F32, kind="ExternalInput")            # padded, scale folded
    bk_d = nc.dram_tensor("bk", [512], F32, kind="ExternalInput")
    wp_d = nc.dram_tensor("wpT", [512, 512], BF16, kind="ExternalInput")     # [padded c, o]
    sel_d = nc.dram_tensor("sel", [128, 128], BF16, kind="ExternalInput")    # denom broadcast selector
    out_d = nc.dram_tensor("out", [C, L], BF16, kind="ExternalOutput")

    with tile.TileContext(nc) as tc:
        with (
            tc.tile_pool(name="singles", bufs=1) as singles,
            tc.tile_pool(name="wt", bufs=12) as wtp,
            tc.tile_pool(name="norm", bufs=3) as normp,
            tc.tile_pool(name="outp", bufs=3) as outp,
            tc.tile_pool(name="psb", bufs=3, space="PSUM") as psb,
            tc.tile_pool(name="psav", bufs=1, space="PSUM") as psav,
        ):
            # ---- load x (full-width rows, 4 queues) and params ----
            xch = [[singles.tile([128, 1024], BF16, name=f"xc{i}_{k}") for k in range(2)]
                   for i in range(4)]
            dma_engs = [nc.sync, nc.gpsimd, nc.scalar]
            for ct in range(4):
                for k in range(2):
                    dma_engs[(2 * ct + k) % 3].dma_start(
                        xch[ct][k][:], x2_d[ct * 128:(ct + 1) * 128, k * 1024:(k + 1) * 1024])
            sel = singles.tile([128, 128], BF16, name="sel")
            nc.scalar.dma_start(sel[:], sel_d[:, :])
            gam = singles.tile([128, 4], F32, name="gam")
            bet = singles.tile([128, 4], F32, name="bet")
            nc.gpsimd.dma_start(gam[:], gamma_d.rearrange("(o p) -> p o", p=128))
            nc.gpsimd.dma_start(bet[:], beta_d.rearrange("(o p) -> p o", p=128))
            bqt = singles.tile([128, 4], F32, name="bqt")
            bkt = singles.tile([128, 4], F32, name="bkt")
            nc.scalar.dma_start(bqt[:], bq_d.rearrange("(o p) -> p o", p=128))
            nc.scalar.dma_start(bkt[:], bk_d.rearrange("(o p) -> p o", p=128))
            wq = [singles.tile([128, 512], BF16, name=f"wq{i}") for i in range(4)]
            wk = [singles.tile([128, 512], BF16, name=f"wk{i}") for i in range(4)]
            wv = [singles.tile([128, 128], BF16, name=f"wv{i}") for i in range(4)]
            wp = [singles.tile([128, 512], BF16, name=f"wp{i}") for i in range(4)]
            for ct in range(4):
                nc.sync.dma_start(wk[ct][:], wk_d[ct * 128:(ct + 1) * 128, :])
                nc.gpsimd.dma_start(wq[ct][:], wq_d[ct * 128:(ct + 1) * 128, :])
                nc.scalar.dma_start(wv[ct][:], wv_d[ct * 128:(ct + 1) * 128, :])

            # vsb ones/zeros after DMA emission so memsets don't block the
            # gpsimd DMA queue ahead of the x2 chunks
            vsb = [singles.tile([128, 16, 32], BF16, name=f"vsb{i}") for i in range(8)]
            for fb in range(8):
                nc.gpsimd.memset(vsb[fb][:, :, 8:9], 1.0)
                nc.gpsimd.memset(vsb[fb][:, :, 9:32], 0.0)

            # ---- BatchNorm stats (per half-chunk as x2 lands) ----
            mv4 = singles.tile([128, 4, 2], F32, name="mv4")
            for ct in range(4):
                stats = normp.tile([128, 4, 6], F32, tag="bnstats")
                for k in range(2):
                    xin = xch[ct][k].rearrange("p (s f) -> p s f", f=512)
                    for si in range(2):
                        nc.vector.bn_stats(out=stats[:, 2 * k + si, :], in_=xin[:, si, :])
                nc.vector.bn_aggr(out=mv4[:, ct, :], in_=stats[:])
            # rstd = sqrt(1/(var+eps)): recip-approx on DVE + ACT Sqrt
            # (Sqrt table load hidden in the idle head)
            r4 = singles.tile([128, 4], F32, name="r4")
            nc.vector.tensor_scalar(out=r4[:], in0=mv4[:, :, 1], scalar1=EPS,
                                    scalar2=None, op0=mybir.AluOpType.add)
            rr4 = singles.tile([128, 4], F32, name="rr4")
            with nc.allow_low_precision(reason="recip-approx 51 ULP, ample for bn rstd"):
                nc.vector.reciprocal_approx_fast(out=rr4[:], in_=r4[:])
            rstd4 = singles.tile([128, 4], F32, name="rstd4")
            nc.scalar.activation(out=rstd4[:], in_=rr4[:],
                                 func=mybir.ActivationFunctionType.Sqrt)
            s4 = singles.tile([128, 4], F32, name="s4")
            nc.vector.tensor_tensor(out=s4[:], in0=rstd4[:], in1=gam[:],
                                    op=mybir.AluOpType.mult)
            tm4 = singles.tile([128, 4], F32, name="tm4")
            nc.vector.tensor_tensor(out=tm4[:], in0=mv4[:, :, 0], in1=s4[:],
                                    op=mybir.AluOpType.mult)
            t4 = singles.tile([128, 4], F32, name="t4")
            nc.vector.tensor_tensor(out=t4[:], in0=bet[:], in1=tm4[:],
                                    op=mybir.AluOpType.subtract)
            # xn = s*x + t (own batch only): split ACT / DVE
            # all on DVE: bf16 in/out tensor_scalar hits the 2x perf mode
            xn = [singles.tile([128, L], BF16, name=f"xn{i}") for i in range(4)]
            for ct in range(4):
                nc.vector.tensor_scalar(out=xn[ct][:],
                                        in0=xch[ct][0][:],
                                        scalar1=s4[:, ct:ct + 1], scalar2=t4[:, ct:ct + 1],
                                        op0=mybir.AluOpType.mult, op1=mybir.AluOpType.add)

            # ---- k/q projections (padded head-block layout) ----
            kT = [singles.tile([128, L], BF16, name=f"kT{i}") for i in range(4)]
            qT = [singles.tile([128, L], BF16, name=f"qT{i}") for i in range(4)]
            kq_state = {}

            def emit_kq_quarter(mo, wi, nh_):
                # wi 0 = k, 1 = q; one nh-half = 4 accumulating MMs (~1.7us,
                # small enough that the exp backlog absorbs the PE insert)
                wmat = (wk, wq)[wi]
                key = (mo, wi)
                if nh_ == 0:
                    kq_state[key] = psb.tile([128, 1024], F32, tag="big", name="ps_kq")
                ps = kq_state[key]
                for kt in range(4):
                    nc.tensor.matmul(
                        ps[:, nh_ * 512:(nh_ + 1) * 512],
                        wmat[kt][:, mo * 128:(mo + 1) * 128],
                        xn[kt][:, nh_ * 512:(nh_ + 1) * 512],
                        start=(kt == 0), stop=(kt == 3))

            def emit_kq_cast(mo, wi):
                # split ACT/DVE so the psb buf frees in ~0.7us and neither
                # exp engine eats the full 1.2us cast
                bias_t, dst = ((bkt, kT), (bqt, qT))[wi]
                ps = kq_state.pop((mo, wi))
                nc.scalar.activation(out=dst[mo][:, 0:512], in_=ps[:, 0:512],
                                     func=mybir.ActivationFunctionType.Identity,
                                     bias=bias_t[:, mo:mo + 1])
                nc.vector.tensor_scalar(out=dst[mo][:, 512:1024], in0=ps[:, 512:1024],
                                        scalar1=bias_t[:, mo:mo + 1], scalar2=None,
                                        op0=mybir.AluOpType.add)

            def emit_kq(mo):
                for wi in range(2):
                    emit_kq_quarter(mo, wi, 0)
                    emit_kq_quarter(mo, wi, 1)
                    emit_kq_cast(mo, wi)

            v_state = {}

            def emit_v_mm(tt):
                # v projection block tt (flipped layout [t, vch]); bv is folded
                # out on the host (attention is affine in v: out += wp@bv)
                ps_full = psb.tile([128, 1024], F32, tag="big", name="psv")
                v_state[tt] = ps_full
                ps = ps_full[:, 0:128]
                for kt in range(4):
                    nc.tensor.matmul(ps[:], xn[kt][:, tt * 128:(tt + 1) * 128],
                                     wv[kt][:], start=(kt == 0), stop=(kt == 3))

            def emit_v_drain(tt):
                ps = v_state.pop(tt)[:, 0:128]
                nc.vector.tensor_copy(
                    vsb[tt][:, :, 0:8], ps.rearrange("p (h d) -> p h d", d=8))

            def emit_v(tt):
                emit_v_mm(tt)
                emit_v_drain(tt)

            emit_kq(0)
            emit_v(0)
            emit_v(1)

            # ---- attention: same-quad AV lag-1, phased normalize ----
            hout = [singles.tile([128, L], BF16, name=f"ho{i}") for i in range(4)]
            avs = {}
            wts = {}

            def emit_score_wave(qd, fb, nh_):
                for jp in range(2):
                    ps = psb.tile([128, 1024], F32, tag="big", name="ps_sc")
                    for side in range(2):
                        j = 2 * jp + side
                        nc.tensor.matmul(
                            ps[:, side * 512:(side + 1) * 512],
                            kT[qd][32 * j:32 * j + 32, fb * 128:(fb + 1) * 128],
                            qT[qd][32 * j:32 * j + 32, nh_ * 512:(nh_ + 1) * 512],
                            tile_position=(32 * j, 0))
                    emit_exp(qd, fb, jp, nh_, ps)

            def emit_exp(qd, fb, jp, nh_, ps):
                # S,A,D,S per fb: split tiles 0/3 free their PSUM buf in
                # ~700ns (rotation), full A/D tiles carry bulk throughput;
                # non-512 splits hit PSUM bank-crossing penalties
                pat = ('S', 'A', 'D', 'S')[(fb * 4 + 2 * nh_ + jp) % 4]
                if pat == 'S':
                    wt = wtp.tile([128, 1024], BF16, tag="wt")
                    nc.scalar.activation(out=wt[:, 0:512], in_=ps[:, 0:512],
                                         func=mybir.ActivationFunctionType.Exp)
                    with nc.allow_low_precision(reason="schraudolph exp approx, validated"):
                        nc.vector.tensor_scalar(
                            out=wt.bitcast(I16)[:, 512:1024], in0=ps[:, 512:1024],
                            scalar1=184.66496, scalar2=16248.75,
                            op0=mybir.AluOpType.mult, op1=mybir.AluOpType.add)
                elif pat == 'D':
                    wti = wtp.tile([128, 1024], I16, tag="wt")
                    with nc.allow_low_precision(reason="schraudolph exp approx, validated"):
                        nc.vector.tensor_scalar(
                            out=wti[:], in0=ps[:],
                            scalar1=184.66496, scalar2=16248.75,
                            op0=mybir.AluOpType.mult, op1=mybir.AluOpType.add)
                    wt = wti.bitcast(BF16)
                else:
                    wt = wtp.tile([128, 1024], BF16, tag="wt")
                    nc.scalar.activation(out=wt[:], in_=ps[:],
                                         func=mybir.ActivationFunctionType.Exp)
                wts[(qd, fb, jp, nh_)] = wt

            def emit_av_half(qd, fb, nh_):
                if (qd, 0) not in avs:
                    avs[(qd, 0)] = psav.tile([128, 1024], F32, tag="av", name="av")
                av = avs[(qd, 0)]
                for j in range(4):
                    jp, side = j // 2, j % 2
                    nc.tensor.matmul(
                        av[32 * j:32 * j + 32, nh_ * 512:(nh_ + 1) * 512],
                        vsb[fb][:, 4 * qd + j, :],
                        wts[(qd, fb, jp, nh_)][:, side * 512:(side + 1) * 512],
                        start=(fb == 0), stop=(fb == 7),
                        tile_position=(0, 32 * j))
                if nh_ == 1:
                    for jp in range(2):
                        for nh2 in range(2):
                            del wts[(qd, fb, jp, nh2)]

            norm_state = {}

            def emit_norm_pool(qd):
                # a_sb copy (frees psav for the selector broadcast)
                av = avs.pop((qd, 0))
                a_sb = normp.tile([128, 1024], BF16, tag="asb")
                nc.scalar.activation(out=a_sb[:, 0:512], in_=av[:, 0:512],
                                     func=mybir.ActivationFunctionType.Copy)
                nc.vector.tensor_copy(a_sb[:, 512:1024], av[:, 512:1024])
                norm_state[qd] = a_sb

            def emit_norm_bcast(qd, last=False):
                # selector matmul broadcasts denom rows (32j+8) to all
                # partitions; full-tile reciprocal; hout = a_sb * recip
                a_sb = norm_state.pop(qd)
                rb_ps = psb.tile([128, 1024], F32, tag="big", name="rbps")
                for nh_ in range(2):
                    nc.tensor.matmul(rb_ps[:, nh_ * 512:(nh_ + 1) * 512],
                                     sel[:], a_sb[:, nh_ * 512:(nh_ + 1) * 512])
                rbr = normp.tile([128, 1024], F32, tag="rbr")
                with nc.allow_low_precision(reason="bf16 softmax denom recip, validated"):
                    nc.vector.reciprocal_approx_fast(out=rbr[:], in_=rb_ps[:])
                if last:
                    # th0 half on DVE first so outproj qd3/th0 can start early
                    nc.vector.tensor_tensor(out=hout[qd][:, 0:512], in0=a_sb[:, 0:512],
                                            in1=rbr[:, 0:512], op=mybir.AluOpType.mult)
                    nc.gpsimd.tensor_tensor(out=hout[qd][:, 512:1024], in0=a_sb[:, 512:1024],
                                            in1=rbr[:, 512:1024], op=mybir.AluOpType.mult)
                else:
                    nc.gpsimd.tensor_tensor(out=hout[qd][:], in0=a_sb[:], in1=rbr[:],
                                            op=mybir.AluOpType.mult)

            # filler schedule: (qd, fb, half) -> list of thunks
            fill = {}

            def add_fill(qd, fb, half, fn):
                fill.setdefault((qd, fb, half), []).append(fn)

            prefill = {}

            def add_prefill(qd, fb, fn):
                prefill.setdefault((qd, fb), []).append(fn)

            for tt in range(2, 8):                          # v(2..7) in quad 0
                add_fill(0, tt - 1, 1, (lambda t: lambda: emit_v_mm(t))(tt))
                add_prefill(0, tt, (lambda t: lambda: emit_v_drain(t))(tt))
            for mo in range(1, 4):                          # kq(1..3) in quads 0..2
                q = mo - 1
                for fb, wi, nh_ in [(2, 0, 0), (3, 0, 1), (5, 1, 0), (6, 1, 1)]:
                    add_fill(q, fb, 0,
                             (lambda m, w, n: lambda: emit_kq_quarter(m, w, n))(mo, wi, nh_))
                for fb, wi in [(4, 0), (7, 1)]:
                    add_prefill(q, fb,
                                (lambda m, w: lambda: emit_kq_cast(m, w))(mo, wi))

            def add_wp_load():
                for ct in range(4):
                    (nc.sync if ct % 2 == 0 else nc.gpsimd).dma_start(
                        wp[ct][:], wp_d[ct * 128:(ct + 1) * 128, :])
            add_fill(1, 4, 0, add_wp_load)

            for q in range(4):
                for fb in range(8):
                    # prefills (kq casts, v drains) run at cycle start so the
                    # psb bufs they hold free before PE needs them, and they
                    # sit ahead of this cycle's exps in the engine queues
                    for fn in prefill.get((q, fb), []):
                        fn()
                    emit_score_wave(q, fb, 0)
                    if fb > 0:
                        if fb == 1 and q > 0:
                            emit_norm_bcast(q - 1)
                        emit_av_half(q, fb - 1, 0)
                    elif q > 0:
                        # av(q-1,7) flush BEHIND the new quad's first scores
                        emit_av_half(q - 1, 7, 0)
                    for fn in fill.get((q, fb, 0), []):
                        fn()
                    emit_score_wave(q, fb, 1)
                    if fb > 0:
                        emit_av_half(q, fb - 1, 1)
                    elif q > 0:
                        emit_av_half(q - 1, 7, 1)
                        emit_norm_pool(q - 1)
                    for fn in fill.get((q, fb, 1), []):
                        fn()

            # ---- tail: AV(3,7), normalize(3), output projection ----
            emit_av_half(3, 7, 0)
            emit_av_half(3, 7, 1)
            emit_norm_pool(3)

            def emit_outproj_mo(mo, ps_full, lo, hi):
                for qd in range(lo, hi):
                    for th in range(2):
                        nc.tensor.matmul(ps_full[:, th * 512:(th + 1) * 512],
                                         wp[qd][:, mo * 128:(mo + 1) * 128],
                                         hout[qd][:, th * 512:(th + 1) * 512],
                                         start=(qd == 0), stop=(qd == 3))

            def finish_outproj(mo, th, ps):
                ot = outp.tile([128, 512], BF16, tag="ot")
                if (2 * mo + th) % 2 == 0:
                    nc.scalar.activation(out=ot[:], in_=ps[:],
                                         func=mybir.ActivationFunctionType.Copy)
                else:
                    nc.vector.tensor_copy(ot[:], ps[:])
                [nc.sync, nc.gpsimd, nc.scalar][(2 * mo + th) % 3].dma_start(
                    out_d[mo * 128:(mo + 1) * 128, th * 512:(th + 1) * 512], ot[:])

            ps_mo0 = psb.tile([128, 1024], F32, tag="big", name="pso")
            emit_outproj_mo(0, ps_mo0, 0, 3)
            emit_norm_bcast(3, last=True)
            ps_mo1 = psb.tile([128, 1024], F32, tag="big", name="pso")
            emit_outproj_mo(1, ps_mo1, 0, 3)
            emit_outproj_mo(0, ps_mo0, 3, 4)
            finish_outproj(0, 0, ps_mo0[:, 0:512])
            finish_outproj(0, 1, ps_mo0[:, 512:1024])
            emit_outproj_mo(1, ps_mo1, 3, 4)
            finish_outproj(1, 0, ps_mo1[:, 0:512])
            finish_outproj(1, 1, ps_mo1[:, 512:1024])
            for mo in range(2, 4):
                ps_full = psb.tile([128, 1024], F32, tag="big", name="pso")
                emit_outproj_mo(mo, ps_full, 0, 4)
                finish_outproj(mo, 0, ps_full[:, 0:512])
                finish_outproj(mo, 1, ps_full[:, 512:1024])

    nc.compile()
    return nc


def _prep_inputs(x, gamma, beta, w_qkv, b_qkv, w_proj, b_proj, num_heads):
    """Shard and lay out inputs for the 8 cores."""
    nh = int(num_heads)
    hd = C // nh
    scale = (3 * nh) ** (-0.5)
    wq_full, wk_full, wv_full = w_qkv[0:C], w_qkv[C:2 * C], w_qkv[2 * C:3 * C]
    bq_full, bk_full, bv_full = b_qkv[0:C], b_qkv[C:2 * C], b_qkv[2 * C:3 * C]

    bf16 = ml_dtypes.bfloat16
    sel = np.zeros((128, 128), np.float32)
    for m in range(128):
        sel[32 * (m // 32) + 8, m] = 1.0
    in_maps = []
    for core in range(N_CORES):
        bi = core // 4
        g = core % 4
        heads = list(range(HEADS_PER_CORE * g, HEADS_PER_CORE * (g + 1)))
        # x2: own batch first, other batch second (BN stats use both)
        x2 = np.concatenate([x[bi], x[1 - bi]], axis=1).astype(bf16)

        # padded q/k weight layouts: [c, 512], col 32*jl + d = channel d*hd + h
        wqT = np.zeros((C, 512), np.float32)
        wkT = np.zeros((C, 512), np.float32)
        bq = np.zeros(512, np.float32)
        bk = np.zeros(512, np.float32)
        wvT = np.zeros((C, 128), np.float32)
        wpT = np.zeros((512, C), np.float32)
        for jl, h in enumerate(heads):
            for d in range(nh):
                ch = d * hd + h
                wqT[:, 32 * jl + d] = wq_full[ch] * scale
                wkT[:, 32 * jl + d] = wk_full[ch]
                bq[32 * jl + d] = bq_full[ch] * scale
                bk[32 * jl + d] = bk_full[ch]
                wvT[:, 8 * jl + d] = wv_full[ch]
                wpT[32 * jl + d, :] = w_proj[:, ch]

        in_maps.append({
            "x2": np.ascontiguousarray(x2),
            "gamma": np.ascontiguousarray(gamma.astype(np.float32)),
            "beta": np.ascontiguousarray(beta.astype(np.float32)),
            "wqT": np.ascontiguousarray(wqT.astype(bf16)),
            "wkT": np.ascontiguousarray(wkT.astype(bf16)),
            "wvT": np.ascontiguousarray(wvT.astype(bf16)),
            "bq": bq, "bk": bk,
            "wpT": np.ascontiguousarray(wpT.astype(bf16)),
            "sel": np.ascontiguousarray(sel.astype(bf16)),
        })
    return in_maps


def kernel(x, gamma, beta, w_qkv, b_qkv, w_proj, b_proj, num_heads, _trace=False):
    x = np.asarray(x, dtype=np.float32)
    gamma = np.asarray(gamma, np.float32)
    beta = np.asarray(beta, np.float32)
    w_qkv = np.asarray(w_qkv, np.float32)
    b_qkv = np.asarray(b_qkv, np.float32)
    w_proj = np.asarray(w_proj, np.float32)
    b_proj = np.asarray(b_proj, np.float32)

    if "nc" not in _CACHE:
        _CACHE["nc"] = _build_nc()
    nc = _CACHE["nc"]
    in_maps = _prep_inputs(x, gamma, beta, w_qkv, b_qkv, w_proj, b_proj, num_heads)
    res = run_bass_kernel_spmd(nc, in_maps, core_ids=list(range(N_CORES)), trace=_trace)
    _CACHE["last_result"] = res

    # bias fold: attention is affine in v (softmax rows sum to 1), so the
    # v-bias flows through as a constant: out += w_proj @ bv + b_proj
    bvh = w_proj @ b_qkv[2 * C:3 * C]
    out = np.zeros((B, C, L), np.float32)
    for bi in range(B):
        acc = np.zeros((C, L), np.float32)
        for g in range(4):
            acc += np.asarray(res.results[bi * 4 + g]["out"]).astype(np.float32)
        out[bi] = acc + x[bi] + (b_proj + bvh)[:, None]
    return out
